# revision 2
# baseline (speedup 1.0000x reference)
"""GNN bi-interaction aggregator for 8 TRN2 NeuronCores — dense formulation.

Reference computation:
    side = entity_embed[src] * att            # [E, D] gather + edge scale
    N_h  = segment_sum(side, dst, N)          # [N, D] scatter-add
    out  = lrelu((x + N_h) @ W1.T + b1) + lrelu((x * N_h) @ W2.T + b2)

Strategy: the gather/scatter is reformulated as a dense matmul against the
att-weighted adjacency.  N_h^T = X^T A^T with A[dst, src] = sum att_e over
edges src->dst.  Nodes (dst rows of A / output) are partitioned across the
8 cores in contiguous slices; A^T is built host-side in bf16 (rel err of
the bf16 formulation vs fp32 reference: ~1.8e-3, well under the 2e-2 gate).

Per core:
  - X (node-major, bf16) and x^T (dst-slice, f32) are SBUF-resident.
  - A^T slice [NSP, NCpc] bf16 streams from HBM in [128 x CJ*448] chunks,
    double-buffered, alternating between the Sync and Activation HWDGE
    queues.
  - PE accumulates N_h^T per 448-wide dst supertile: 391 chained matmuls
    (stationary = 128x128 X tile, moving = 128x448 A^T block).
  - Epilogue per supertile: sum/prod on DVE, two weight matmuls, LeakyReLU
    via two ACT passes each (relu(z+b) - 0.01*relu(-(z+b))), final add on
    DVE, DMA out.  Fully pipelined across supertiles.

This puts the kernel at the bf16 roofline ridge: per core ~0.63 GB of A^T
streamed (~1 ms at full DMA bw) against ~40 GMAC on PE (~1 ms at bf16 peak).

Env knobs (for benchmarking only):
  KREP  — repeat the whole compute R times inside one program (timing).
  DSTAGE— -1 consts only / 1 A-stream only / 2 PE only / 3 DMA+PE / 4 full.
"""

import math
import os
from contextlib import ExitStack
from dataclasses import dataclass

import ml_dtypes
import numpy as np

import concourse.bacc as bacc
import concourse.bass as bass
import concourse.mybir as mybir
from concourse.bass_utils import run_bass_kernel_spmd

F32 = mybir.dt.float32
BF16 = mybir.dt.bfloat16
LANES = 128
D = 128
SW = 448   # supertile width: dst columns per PSUM accumulation group
CJ = 17    # j-tiles (of 128 src rows) per A^T DMA chunk


# --------------------------------------------------------------------------
# Host-side planning
# --------------------------------------------------------------------------

@dataclass
class Plan:
    n_nodes: int
    n_cores: int
    NSP: int    # padded source-node count (NJ * 128)
    NJ: int     # source tiles
    NCpc: int   # dst columns per core (NSUP * SW)
    NSUP: int   # supertiles per core
    CPS: int    # A^T DMA chunks per supertile
    inputs: list


def make_plan(entity_embed, att, src, dst, n_cores=8):
    n, d = entity_embed.shape
    assert d == D
    NSP = ((n + LANES - 1) // LANES) * LANES
    NJ = NSP // LANES
    NCpc = ((-(-n // n_cores)) + SW - 1) // SW * SW
    NSUP = NCpc // SW
    CPS = (NJ + CJ - 1) // CJ
    NPAD = NCpc * n_cores

    src = np.asarray(src, np.int64)
    dst = np.asarray(dst, np.int64)
    attf = np.asarray(att, np.float32).reshape(-1)

    # merge duplicate (dst, src) pairs so the scatter below can assign
    key = dst * NSP + src
    uk, inv = np.unique(key, return_inverse=True)
    asum = np.bincount(inv, weights=attf.astype(np.float64)).astype(np.float32)
    usrc = uk % NSP
    udst = uk // NSP
    a16 = np.asarray(asum.astype(ml_dtypes.bfloat16)).view(np.uint16)

    X = np.zeros((NSP, D), np.float32)
    X[:n] = np.asarray(entity_embed, np.float32)
    Xb = X.astype(ml_dtypes.bfloat16)
    # node-major stationary layout: XN[p, j*128 + f] = X[j*128 + p, f]
    XN = np.ascontiguousarray(
        Xb.reshape(NJ, LANES, D).transpose(1, 0, 2).reshape(LANES, NJ * D))
    Xp = np.zeros((NPAD, D), np.float32)
    Xp[:n] = X[:n]

    inputs = []
    for c in range(n_cores):
        lo, hi = c * NCpc, (c + 1) * NCpc
        m = (udst >= lo) & (udst < hi)
        ATu = np.zeros(NSP * NCpc, np.uint16)
        ATu[usrc[m] * NCpc + (udst[m] - lo)] = a16[m]
        AT = ATu.view(ml_dtypes.bfloat16).reshape(NSP, NCpc)
        xTc = np.ascontiguousarray(Xp[lo:hi].T)  # [128, NCpc] f32
        inputs.append({"AT": AT, "XN": XN, "xTc": xTc})

    return Plan(n_nodes=n, n_cores=n_cores, NSP=NSP, NJ=NJ, NCpc=NCpc,
                NSUP=NSUP, CPS=CPS, inputs=inputs)


# --------------------------------------------------------------------------
# Bass program
# --------------------------------------------------------------------------

def build_nc(plan, debug=False):
    p = plan
    STAGE = int(os.environ.get("DSTAGE", "4"))
    KREP = int(os.environ.get("KREP", "1"))
    NJ, NSUP, CPS, NCpc = p.NJ, p.NSUP, p.CPS, p.NCpc
    NGG = KREP * NSUP
    AF = mybir.ActivationFunctionType

    nc = bacc.Bacc("TRN2", target_bir_lowering=False, debug=debug)

    AT_d = nc.dram_tensor("AT", [p.NSP, NCpc], BF16, kind="ExternalInput")
    XN_d = nc.dram_tensor("XN", [LANES, NJ * D], BF16, kind="ExternalInput")
    xTc_d = nc.dram_tensor("xTc", [LANES, NCpc], F32, kind="ExternalInput")
    w1_d = nc.dram_tensor("W1t", [D, D], F32, kind="ExternalInput")
    w2_d = nc.dram_tensor("W2t", [D, D], F32, kind="ExternalInput")
    b1_d = nc.dram_tensor("b1c", [D, 1], F32, kind="ExternalInput")
    b2_d = nc.dram_tensor("b2c", [D, 1], F32, kind="ExternalInput")
    b1n_d = nc.dram_tensor("b1n", [D, 1], F32, kind="ExternalInput")
    b2n_d = nc.dram_tensor("b2n", [D, 1], F32, kind="ExternalInput")
    outT_d = nc.dram_tensor("outT", [LANES, NCpc], F32, kind="ExternalOutput")

    with ExitStack() as ctx:
        sb = lambda name, shape, dt=F32: ctx.enter_context(
            nc.sbuf_tensor(name, shape, dt))
        ps = lambda name, shape: ctx.enter_context(
            nc.psum_tensor(name, shape, F32))
        sem = lambda name: ctx.enter_context(nc.semaphore(name))

        XN = sb("XN_sb", [LANES, NJ * D], BF16)
        xTc = sb("xTc_sb", [LANES, NCpc])
        abuf = [sb(f"abuf{i}", [LANES, CJ * SW], BF16) for i in range(2)]
        w1s = sb("w1_sb", [D, D])
        w2s = sb("w2_sb", [D, D])
        b1s = sb("b1_sb", [D, 1])
        b2s = sb("b2_sb", [D, 1])
        b1ns = sb("b1n_sb", [D, 1])
        b2ns = sb("b2n_sb", [D, 1])
        sumb = [sb(f"sum{i}", [LANES, SW]) for i in range(2)]
        prodb = [sb(f"prod{i}", [LANES, SW]) for i in range(2)]
        t1b = sb("t1", [LANES, SW])
        u1b = sb("u1", [LANES, SW])
        t2b = sb("t2", [LANES, SW])
        u2b = sb("u2", [LANES, SW])
        wb = sb("wb", [LANES, SW])
        outb = [sb(f"outb{i}", [LANES, SW]) for i in range(2)]

        acc = [ps(f"acc{i}", [LANES, SW]) for i in range(2)]
        zb1 = ps("zb1", [LANES, SW])
        zb2 = ps("zb2", [LANES, SW])

        c16 = sem("c16")
        admaS = sem("admaS")   # even-gid A chunks (sync queue)
        admaA = sem("admaA")   # odd-gid A chunks (scalar queue)
        mmch = sem("mmch")     # PE: chunks consumed
        spsem = sem("spsem")   # DVE: sum/prod per super
        wz = sem("wz")         # PE: weight matmuls per super
        wlr = sem("wlr")       # ACT: lrelu per super
        wadd = sem("wadd")     # DVE: final add per super
        osem = sem("osem")     # out DMAs

        const_loads = [
            (XN, XN_d), (xTc, xTc_d), (w1s, w1_d), (w2s, w2_d),
            (b1s, b1_d), (b2s, b2_d), (b1ns, b1n_d), (b2ns, b2n_d),
        ]
        NCONST = len(const_loads)

        def chunk_geom(k):
            j0 = k * CJ
            return j0, min(CJ, NJ - j0)

        def at_ap(j0, cs, i):
            base = AT_d[:, :]
            return bass.AP(
                tensor=base.tensor,
                offset=j0 * LANES * NCpc + i * SW,
                ap=[[NCpc, LANES], [LANES * NCpc, cs], [1, SW]],
            )

        stream_a = STAGE in (1, 3, 4)
        run_pe = STAGE in (2, 3, 4)

        with nc.Block() as block:

            @block.sync
            def _(sync):
                for dst_sb, src_d in const_loads:
                    sync.dma_start(dst_sb[:], src_d[:]).then_inc(c16, 16)
                if STAGE == -1:
                    sync.wait_ge(c16, 16 * NCONST)
                    return
                scnt = 0
                for gg in range(NGG):
                    i = gg % NSUP
                    for k in range(CPS):
                        gid = gg * CPS + k
                        if gid % 2 or not stream_a:
                            continue
                        if run_pe and gid >= 2:
                            sync.wait_ge(mmch, gid - 1)
                        j0, cs = chunk_geom(k)
                        sync.dma_start(
                            abuf[0][:, 0:cs * SW], at_ap(j0, cs, i),
                        ).then_inc(admaS, 16)
                        scnt += 1
                    if STAGE >= 4 and gg >= 1:
                        sync.wait_ge(wadd, gg)
                        g = gg - 1
                        sync.dma_start(
                            outT_d[:, (g % NSUP) * SW:(g % NSUP + 1) * SW],
                            outb[g % 2][:, :],
                        ).then_inc(osem, 16)
                if STAGE >= 4:
                    sync.wait_ge(wadd, NGG)
                    g = NGG - 1
                    sync.dma_start(
                        outT_d[:, (g % NSUP) * SW:(g % NSUP + 1) * SW],
                        outb[g % 2][:, :],
                    ).then_inc(osem, 16)
                    sync.wait_ge(osem, 16 * NGG)
                elif stream_a:
                    sync.wait_ge(admaS, 16 * scnt)

            @block.scalar
            def _(scalar):
                if STAGE == -1:
                    return
                scalar.wait_ge(c16, 16 * NCONST)

                def lrelu(g):
                    scalar.wait_ge(wz, g + 1)
                    if g >= 1:
                        scalar.wait_ge(wadd, g)
                    nc.scalar.activation(
                        t1b[:, :], zb1[:, :], AF.Relu,
                        bias=b1s[:, 0:1], scale=1.0)
                    nc.scalar.activation(
                        u1b[:, :], zb1[:, :], AF.Relu,
                        bias=b1ns[:, 0:1], scale=-0.01)
                    nc.scalar.activation(
                        t2b[:, :], zb2[:, :], AF.Relu,
                        bias=b2s[:, 0:1], scale=1.0)
                    nc.scalar.activation(
                        u2b[:, :], zb2[:, :], AF.Relu,
                        bias=b2ns[:, 0:1], scale=-0.01,
                    ).then_inc(wlr, 1)

                acnt = 0
                for gg in range(NGG):
                    i = gg % NSUP
                    for k in range(CPS):
                        gid = gg * CPS + k
                        if gid % 2 == 0 or not stream_a:
                            continue
                        if run_pe and gid >= 2:
                            scalar.wait_ge(mmch, gid - 1)
                        j0, cs = chunk_geom(k)
                        scalar.dma_start(
                            abuf[1][:, 0:cs * SW], at_ap(j0, cs, i),
                        ).then_inc(admaA, 16)
                        acnt += 1
                    if STAGE >= 4 and gg >= 1:
                        lrelu(gg - 1)
                if STAGE >= 4:
                    lrelu(NGG - 1)
                elif stream_a and acnt:
                    scalar.wait_ge(admaA, 16 * acnt)

            @block.tensor
            def _(tensor):
                if not run_pe:
                    return
                tensor.wait_ge(c16, 16 * NCONST)

                def zmm(g):
                    tensor.wait_ge(spsem, g + 1)
                    if g >= 1:
                        tensor.wait_ge(wlr, g)
                    nc.tensor.matmul(
                        zb1[:, :], w1s[:, :], sumb[g % 2][:, :],
                        start=True, stop=True)
                    nc.tensor.matmul(
                        zb2[:, :], w2s[:, :], prodb[g % 2][:, :],
                        start=True, stop=True).then_inc(wz, 1)

                sc = ac = 0
                for gg in range(NGG):
                    if STAGE >= 4 and gg >= 2:
                        tensor.wait_ge(spsem, gg - 1)
                    for k in range(CPS):
                        gid = gg * CPS + k
                        j0, cs = chunk_geom(k)
                        if stream_a:
                            if gid % 2 == 0:
                                sc += 1
                                tensor.wait_ge(admaS, 16 * sc)
                            else:
                                ac += 1
                                tensor.wait_ge(admaA, 16 * ac)
                        b = abuf[gid % 2]
                        for jj in range(cs):
                            j = j0 + jj
                            mm = nc.tensor.matmul(
                                acc[gg % 2][:, :],
                                XN[:, j * D:(j + 1) * D],
                                b[:, jj * SW:(jj + 1) * SW],
                                start=(j == 0), stop=(j == NJ - 1),
                            )
                        mm.then_inc(mmch, 1)
                    if STAGE >= 4 and gg >= 1:
                        zmm(gg - 1)
                if STAGE >= 4:
                    zmm(NGG - 1)

            @block.vector
            def _(vector):
                if STAGE < 4:
                    return
                vector.wait_ge(c16, 16 * NCONST)

                def sum_prod(gg):
                    i = gg % NSUP
                    vector.wait_ge(mmch, (gg + 1) * CPS)
                    if gg >= 2:
                        vector.wait_ge(wz, gg - 1)
                    xs = xTc[:, i * SW:(i + 1) * SW]
                    nc.vector.tensor_tensor(
                        sumb[gg % 2][:, :], acc[gg % 2][:, :], xs,
                        mybir.AluOpType.add)
                    nc.vector.tensor_tensor(
                        prodb[gg % 2][:, :], acc[gg % 2][:, :], xs,
                        mybir.AluOpType.mult).then_inc(spsem, 1)

                def final_add(gg):
                    vector.wait_ge(wlr, gg + 1)
                    if gg >= 2:
                        vector.wait_ge(osem, 16 * (gg - 1))
                    nc.vector.tensor_tensor(
                        outb[gg % 2][:, :], t1b[:, :], t2b[:, :],
                        mybir.AluOpType.add)
                    nc.vector.tensor_tensor(
                        wb[:, :], u1b[:, :], u2b[:, :],
                        mybir.AluOpType.add)
                    nc.vector.drain()
                    nc.vector.tensor_tensor(
                        outb[gg % 2][:, :], outb[gg % 2][:, :], wb[:, :],
                        mybir.AluOpType.subtract)
                    nc.vector.drain().then_inc(wadd, 1)

                for gg in range(NGG):
                    if gg >= 1:
                        final_add(gg - 1)
                    sum_prod(gg)
                final_add(NGG - 1)

    nc.compile()
    return nc


# --------------------------------------------------------------------------
# Entry point
# --------------------------------------------------------------------------

def make_consts(W1, b1, W2, b2):
    return {
        "W1t": np.ascontiguousarray(np.asarray(W1, np.float32).T),
        "W2t": np.ascontiguousarray(np.asarray(W2, np.float32).T),
        "b1c": np.asarray(b1, np.float32).reshape(D, 1).copy(),
        "b2c": np.asarray(b2, np.float32).reshape(D, 1).copy(),
        "b1n": (-0.01 * np.asarray(b1, np.float32)).reshape(D, 1).copy(),
        "b2n": (-0.01 * np.asarray(b2, np.float32)).reshape(D, 1).copy(),
    }


def _run(plan, W1, b1, W2, b2, n_cores, debug=False, trace=False):
    nc = build_nc(plan, debug=debug)
    consts = make_consts(W1, b1, W2, b2)
    in_maps = []
    for c in range(n_cores):
        m = dict(plan.inputs[c])
        m.update(consts)
        in_maps.append(m)
    return run_bass_kernel_spmd(nc, in_maps, core_ids=list(range(n_cores)),
                                trace=trace)


def assemble_output(plan, results):
    outs = [np.asarray(results[c]["outT"]).T for c in range(plan.n_cores)]
    return np.concatenate(outs, axis=0)[:plan.n_nodes]


def kernel(entity_embed, att, W1, b1, W2, b2, src, dst):
    entity_embed = np.asarray(entity_embed, np.float32)
    att = np.asarray(att, np.float32)
    src = np.asarray(src).astype(np.int64)
    dst = np.asarray(dst).astype(np.int64)
    plan = make_plan(entity_embed, att, src, dst, n_cores=8)
    res = _run(plan, W1, b1, W2, b2, n_cores=8)
    return assemble_output(plan, res.results)


if __name__ == "__main__":
    pass


# revision 16
# speedup vs baseline: 3.2207x; 3.2207x over previous
"""GNN bi-interaction aggregator for 8 TRN2 NeuronCores — dense formulation.

Reference computation:
    side = entity_embed[src] * att            # [E, D] gather + edge scale
    N_h  = segment_sum(side, dst, N)          # [N, D] scatter-add
    out  = lrelu((x + N_h) @ W1.T + b1) + lrelu((x * N_h) @ W2.T + b2)

Strategy: the gather/scatter is reformulated as a dense matmul against the
att-weighted adjacency.  N_h^T = X^T A^T with A[dst, src] = sum att_e over
edges src->dst.  Destination nodes are partitioned across the 8 cores in
contiguous slices; A^T is built host-side in bf16 (rel err of the bf16
formulation vs fp32 reference ~1.8e-3, well under the 2e-2 gate).

Per core:
  - X (node-major bf16, the matmul stationary) and x^T (dst slice, f32)
    are SBUF-resident.
  - A^T slice [NSP, NCpc] bf16 streams from HBM in chunks of CJ j-tiles x
    (S*SW) dst columns (3136B contiguous runs), multi-buffered across the
    Sync / Activation HWDGE queues (optionally gpsimd SWDGE as a third).
  - PE: supertiles are processed in groups of S=4 sharing one stationary
    load per source tile: ldweights(X_j) + S no-load matmuls into S PSUM
    banks (explicit InstLdweights + InstMatmult(ldweights=False) — ~3x
    faster than self-loading matmuls on TRN2).
  - Epilogue per supertile: sum/prod on DVE (reads PSUM), two weight
    matmuls, LeakyReLU as relu(z+b) - 0.01*relu(-(z+b)) on ACT, final
    combine on DVE, DMA out.  Pipelined across groups.

Env knobs (benchmarking only):
  KREP   — repeat the whole compute R times inside one program (timing).
  DSTAGE — -1 consts only / 1 A-stream only / 2 PE only / 4 full.
  KSELF  — 1: use self-loading matmuls (no ldweights sharing) fallback.
  KNQ    — number of A-stream DMA queues (2 default, 3 adds gpsimd).
  KSW/KS/KCJ — supertile width / group size / j-tiles per chunk.
"""

import math
import os
from contextlib import ExitStack
from dataclasses import dataclass

import ml_dtypes
import numpy as np

import concourse.bacc as bacc
import concourse.bass as bass
import concourse.mybir as mybir
from concourse.bass_utils import run_bass_kernel_spmd

F32 = mybir.dt.float32
BF16 = mybir.dt.bfloat16
LANES = 128
D = 128


# --------------------------------------------------------------------------
# Host-side planning
# --------------------------------------------------------------------------

@dataclass
class Plan:
    n_nodes: int
    n_cores: int
    NSP: int    # padded source-node count (NJ * 128)
    NJ: int     # source tiles
    NCpc: int   # dst columns per core (NSUP * SW)
    NSUP: int   # supertiles per core
    S: int      # supertiles per PE group (PSUM banks used)
    NG: int     # groups per core (NSUP / S)
    CPS: int    # A^T DMA chunks per group
    SW: int
    CJ: int
    inputs: list


def make_plan(entity_embed, att, src, dst, n_cores=8):
    n, d = entity_embed.shape
    assert d == D
    SW = int(os.environ.get("KSW", "392"))
    S = int(os.environ.get("KS", "4"))
    NQ = int(os.environ.get("KNQ", "3"))
    CJ = int(os.environ.get("KCJ", "8" if NQ == 2 else "6"))
    NSP = ((n + LANES - 1) // LANES) * LANES
    NJ = NSP // LANES
    gw = SW * S
    NCpc = ((-(-n // n_cores)) + gw - 1) // gw * gw
    NSUP = NCpc // SW
    NG = NSUP // S
    CPS = (NJ + CJ - 1) // CJ

    src = np.asarray(src, np.int64)
    dst = np.asarray(dst, np.int64)
    attf = np.asarray(att, np.float32).reshape(-1)

    # merge duplicate (dst, src) pairs so the scatter below can assign
    key = dst * NSP + src
    uk, inv = np.unique(key, return_inverse=True)
    asum = np.bincount(inv, weights=attf.astype(np.float64)).astype(np.float32)
    usrc = uk % NSP
    udst = uk // NSP
    a16 = np.asarray(asum.astype(ml_dtypes.bfloat16)).view(np.uint16)

    X = np.zeros((NSP, D), np.float32)
    X[:n] = np.asarray(entity_embed, np.float32)
    Xb = X.astype(ml_dtypes.bfloat16)
    # node-major stationary layout: XN[p, j*128 + f] = X[j*128 + p, f]
    XN = np.ascontiguousarray(
        Xb.reshape(NJ, LANES, D).transpose(1, 0, 2).reshape(LANES, NJ * D))
    NPAD = NCpc * n_cores
    Xp = np.zeros((NPAD, D), np.float32)
    Xp[:n] = X[:n]

    inputs = []
    for c in range(n_cores):
        lo, hi = c * NCpc, (c + 1) * NCpc
        m = (udst >= lo) & (udst < hi)
        ATu = np.zeros(NSP * NCpc, np.uint16)
        ATu[usrc[m] * NCpc + (udst[m] - lo)] = a16[m]
        AT = ATu.view(ml_dtypes.bfloat16).reshape(NSP, NCpc)
        xTc = np.ascontiguousarray(Xp[lo:hi].T)  # [128, NCpc] f32
        inputs.append({"AT": AT, "XN": XN, "xTc": xTc})

    return Plan(n_nodes=n, n_cores=n_cores, NSP=NSP, NJ=NJ, NCpc=NCpc,
                NSUP=NSUP, S=S, NG=NG, CPS=CPS, SW=SW, CJ=CJ, inputs=inputs)


# --------------------------------------------------------------------------
# Bass program
# --------------------------------------------------------------------------

def build_nc(plan, debug=False):
    p = plan
    STAGE = int(os.environ.get("DSTAGE", "4"))
    KREP = int(os.environ.get("KREP", "1"))
    KSELF = int(os.environ.get("KSELF", "0"))
    NQ = int(os.environ.get("KNQ", "3"))
    NJ, NCpc, NSUP, S, NG, CPS = p.NJ, p.NCpc, p.NSUP, p.S, p.NG, p.CPS
    SW, CJ = p.SW, p.CJ
    GW = S * SW
    NGG = KREP * NG          # total groups
    NUU = KREP * NSUP        # total supertiles
    TOTCH = NGG * CPS        # total A chunks
    AF = mybir.ActivationFunctionType

    nc = bacc.Bacc("TRN2", target_bir_lowering=False, debug=debug)

    AT_d = nc.dram_tensor("AT", [p.NSP, NCpc], BF16, kind="ExternalInput")
    XN_d = nc.dram_tensor("XN", [LANES, NJ * D], BF16, kind="ExternalInput")
    xTc_d = nc.dram_tensor("xTc", [LANES, NCpc], F32, kind="ExternalInput")
    w1_d = nc.dram_tensor("W1t", [D, D], F32, kind="ExternalInput")
    w2_d = nc.dram_tensor("W2t", [D, D], F32, kind="ExternalInput")
    b1_d = nc.dram_tensor("b1c", [D, 1], F32, kind="ExternalInput")
    b2_d = nc.dram_tensor("b2c", [D, 1], F32, kind="ExternalInput")
    b1n_d = nc.dram_tensor("b1n", [D, 1], F32, kind="ExternalInput")
    b2n_d = nc.dram_tensor("b2n", [D, 1], F32, kind="ExternalInput")
    outT_d = nc.dram_tensor("outT", [LANES, NCpc], F32, kind="ExternalOutput")

    with ExitStack() as ctx:
        sb = lambda name, shape, dt=F32: ctx.enter_context(
            nc.sbuf_tensor(name, shape, dt))
        ps = lambda name, shape: ctx.enter_context(
            nc.psum_tensor(name, shape, F32))
        sem = lambda name: ctx.enter_context(nc.semaphore(name))

        XN = sb("XN_sb", [LANES, NJ * D], BF16)
        xTc = sb("xTc_sb", [LANES, NCpc])
        abuf = [sb(f"abuf{i}", [LANES, CJ * GW], BF16) for i in range(NQ)]
        w1s = sb("w1_sb", [D, D])
        w2s = sb("w2_sb", [D, D])
        b1s = sb("b1_sb", [D, 1])
        b2s = sb("b2_sb", [D, 1])
        b1ns = sb("b1n_sb", [D, 1])
        b2ns = sb("b2n_sb", [D, 1])
        sumb = [sb(f"sum{i}", [LANES, SW]) for i in range(2)]
        prodb = [sb(f"prod{i}", [LANES, SW]) for i in range(2)]
        t1b = sb("t1", [LANES, SW])
        u1b = sb("u1", [LANES, SW])
        t2b = sb("t2", [LANES, SW])
        u2b = sb("u2", [LANES, SW])
        wb = sb("wb", [LANES, SW])
        outb = [sb(f"outb{i}", [LANES, SW]) for i in range(2)]

        acc = [ps(f"acc{i}", [LANES, SW]) for i in range(S)]
        zb1 = [ps(f"zb1_{i}", [LANES, SW]) for i in range(2)]
        zb2 = [ps(f"zb2_{i}", [LANES, SW]) for i in range(2)]

        c16 = sem("c16")
        adma = [sem(f"adma{q}") for q in range(NQ)]
        mmch = sem("mmch")     # PE: chunks consumed
        spsem = sem("spsem")   # DVE: sum/prod per super
        wz = sem("wz")         # PE: weight matmuls per super
        wlr = sem("wlr")       # ACT: lrelu per super
        wadd = sem("wadd")     # DVE: final add per super
        osem = [sem("osem0"), sem("osem1")]  # out DMAs (parity split)

        const_loads = [
            (XN, XN_d), (xTc, xTc_d), (w1s, w1_d), (w2s, w2_d),
            (b1s, b1_d), (b2s, b2_d), (b1ns, b1n_d), (b2ns, b2n_d),
        ]
        NCONST = len(const_loads)

        def chunk_geom(k):
            j0 = k * CJ
            return j0, min(CJ, NJ - j0)

        def at_ap(j0, cs, grp):
            base = AT_d[:, :]
            return bass.AP(
                tensor=base.tensor,
                offset=j0 * LANES * NCpc + grp * GW,
                ap=[[NCpc, LANES], [LANES * NCpc, cs], [1, GW]],
            )

        def mm_noload(out, lhsT, rhs, start, stop):
            """InstMatmult with ldweights=False — reuses the PE array's
            currently loaded stationary (paired with nc.tensor.ldweights)."""
            eng = nc.tensor
            ifmap_ap = eng.lower_ap(rhs.opt({0}), opt=False)
            weights_ap = eng.lower_ap(
                lhsT.opt({0}), opt=False, for_matmul_weights=True)
            out_ap = eng.lower_ap(out)
            return eng.add_instruction(mybir.InstMatmult(
                name=eng.bass.get_next_instruction_name(),
                replication_resolution=0,
                replication_shift_amnt=0,
                replication_num_rows=0,
                start_tensor_calc=start,
                stop_tensor_calc=stop,
                ins=[ifmap_ap, weights_ap],
                outs=[out_ap],
                tile_position=(0, 0),
                tile_size=(128, 128),
                ldweights=False,
            ))

        stream_a = STAGE in (1, 4)
        run_pe = STAGE in (2, 4)

        # chunk gid -> queue: round robin over NQ; engine: q0=sync, q1=scalar,
        # q2=gpsimd.  Buffer = gid % NQ (buffer matches queue, FIFO per queue).
        def dma_all(eng, q):
            """Emit this queue's share of A-chunk DMAs."""
            cnt = 0
            for G in range(NGG):
                grp = G % NG
                for k in range(CPS):
                    gid = G * CPS + k
                    if gid % NQ != q or not stream_a:
                        continue
                    if run_pe and gid >= NQ:
                        eng.wait_ge(mmch, gid - (NQ - 1))
                    j0, cs = chunk_geom(k)
                    eng.dma_start(
                        abuf[q][:, 0:cs * GW], at_ap(j0, cs, grp),
                    ).then_inc(adma[q], 16)
                    cnt += 1
            if stream_a and cnt and not run_pe:
                eng.wait_ge(adma[q], 16 * cnt)

        with nc.Block() as block:

            @block.sync
            def _(sync):
                for dst_sb, src_d in const_loads:
                    sync.dma_start(dst_sb[:], src_d[:]).then_inc(c16, 16)
                if STAGE == -1:
                    sync.wait_ge(c16, 16 * NCONST)
                    return
                dma_all(sync, 0)

            @block.scalar
            def _(scalar):
                if STAGE == -1:
                    return
                scalar.wait_ge(c16, 16 * NCONST)

                def lrelu(u):
                    scalar.wait_ge(wz, u + 1)
                    if u >= 1:
                        scalar.wait_ge(wadd, u)
                    z1, z2 = zb1[u % 2], zb2[u % 2]
                    nc.scalar.activation(
                        t1b[:, :], z1[:, :], AF.Relu,
                        bias=b1s[:, 0:1], scale=1.0)
                    nc.scalar.activation(
                        u1b[:, :], z1[:, :], AF.Relu,
                        bias=b1ns[:, 0:1], scale=-0.01)
                    nc.scalar.activation(
                        t2b[:, :], z2[:, :], AF.Relu,
                        bias=b2s[:, 0:1], scale=1.0)
                    nc.scalar.activation(
                        u2b[:, :], z2[:, :], AF.Relu,
                        bias=b2ns[:, 0:1], scale=-0.01,
                    ).then_inc(wlr, 1)

                if STAGE >= 4:
                    # lrelus of group G-1 must precede group G's chunk
                    # issues: the issue guards need PE progress into group
                    # G, which needs the G-1 epilogue (acc banks reused).
                    for G in range(NGG):
                        grp = G % NG
                        if G >= 1:
                            for s in range(S):
                                lrelu((G - 1) * S + s)
                        for k in range(CPS):
                            gid = G * CPS + k
                            if gid % NQ != 1 or not stream_a:
                                continue
                            if run_pe and gid >= NQ:
                                scalar.wait_ge(mmch, gid - (NQ - 1))
                            j0, cs = chunk_geom(k)
                            scalar.dma_start(
                                abuf[1][:, 0:cs * GW], at_ap(j0, cs, grp),
                            ).then_inc(adma[1], 16)
                    for s in range(S):
                        lrelu((NGG - 1) * S + s)
                else:
                    dma_all(scalar, 1)

            @block.gpsimd
            def _(gpsimd):
                """Output DMAs (own SWDGE queue — decoupled from the A
                stream so no ordering cycles), plus queue 2 of the A stream
                when NQ=3."""
                if STAGE == -1:
                    return
                if STAGE < 4 and NQ < 3:
                    return
                gpsimd.wait_ge(c16, 16 * NCONST)

                def outdma(u):
                    gpsimd.wait_ge(wadd, u + 1)
                    i = u % NSUP
                    gpsimd.dma_start(
                        outT_d[:, i * SW:(i + 1) * SW],
                        outb[u % 2][:, :],
                    ).then_inc(osem[u % 2], 16)

                if NQ < 3:
                    for u in range(NUU):
                        outdma(u)
                else:
                    cnt = 0
                    for G in range(NGG):
                        grp = G % NG
                        if STAGE >= 4 and G >= 1:
                            for s in range(S):
                                outdma((G - 1) * S + s)
                        for k in range(CPS):
                            gid = G * CPS + k
                            if gid % NQ != 2 or not stream_a:
                                continue
                            if run_pe and gid >= NQ:
                                gpsimd.wait_ge(mmch, gid - (NQ - 1))
                            j0, cs = chunk_geom(k)
                            gpsimd.dma_start(
                                abuf[2][:, 0:cs * GW], at_ap(j0, cs, grp),
                            ).then_inc(adma[2], 16)
                            cnt += 1
                    if STAGE >= 4:
                        for s in range(S):
                            outdma((NGG - 1) * S + s)
                    elif stream_a and cnt and not run_pe:
                        gpsimd.wait_ge(adma[2], 16 * cnt)
                if STAGE >= 4:
                    gpsimd.wait_ge(osem[0], 16 * ((NUU + 1) // 2))
                    if NUU > 1:
                        gpsimd.wait_ge(osem[1], 16 * (NUU // 2))

            @block.tensor
            def _(tensor):
                if not run_pe:
                    return
                tensor.wait_ge(c16, 16 * NCONST)

                def zmm(u):
                    tensor.wait_ge(spsem, u + 1)
                    if u >= 2:
                        tensor.wait_ge(wlr, u - 1)
                    nc.tensor.matmul(
                        zb1[u % 2][:, :], w1s[:, :], sumb[u % 2][:, :],
                        start=True, stop=True)
                    nc.tensor.matmul(
                        zb2[u % 2][:, :], w2s[:, :], prodb[u % 2][:, :],
                        start=True, stop=True).then_inc(wz, 1)

                qcnt = [0] * NQ
                for G in range(NGG):
                    if STAGE >= 4 and G >= 1:
                        tensor.wait_ge(spsem, G * S)
                    for k in range(CPS):
                        gid = G * CPS + k
                        j0, cs = chunk_geom(k)
                        if stream_a:
                            q = gid % NQ
                            qcnt[q] += 1
                            tensor.wait_ge(adma[q], 16 * qcnt[q])
                        b = abuf[gid % NQ]
                        for jj in range(cs):
                            j = j0 + jj
                            st = (j == 0)
                            sp = (j == NJ - 1)
                            if KSELF:
                                for s in range(S):
                                    mm = nc.tensor.matmul(
                                        acc[s][:, :],
                                        XN[:, j * D:(j + 1) * D],
                                        b[:, (jj * S + s) * SW:
                                          (jj * S + s + 1) * SW],
                                        start=st, stop=sp)
                            else:
                                nc.tensor.ldweights(XN[:, j * D:(j + 1) * D])
                                for s in range(S):
                                    mm = mm_noload(
                                        acc[s][:, :],
                                        XN[:, j * D:(j + 1) * D],
                                        b[:, (jj * S + s) * SW:
                                          (jj * S + s + 1) * SW],
                                        start=st, stop=sp)
                        mm.then_inc(mmch, 1)
                    if STAGE >= 4:
                        for s in range(S):
                            zmm(G * S + s)

            @block.vector
            def _(vector):
                if STAGE < 4:
                    return
                vector.wait_ge(c16, 16 * NCONST)

                def sum_prod(u):
                    G = u // S
                    i = u % NSUP
                    vector.wait_ge(mmch, (G + 1) * CPS)
                    if u >= 2:
                        vector.wait_ge(wz, u - 1)
                    xs = xTc[:, i * SW:(i + 1) * SW]
                    a = acc[u % S]
                    nc.vector.tensor_tensor(
                        sumb[u % 2][:, :], a[:, :], xs,
                        mybir.AluOpType.add)
                    nc.vector.tensor_tensor(
                        prodb[u % 2][:, :], a[:, :], xs,
                        mybir.AluOpType.mult).then_inc(spsem, 1)

                def final_add(u):
                    vector.wait_ge(wlr, u + 1)
                    if u >= 2:
                        # out DMA of super u-2 (same outb parity) must be done
                        vector.wait_ge(osem[u % 2], 16 * (u // 2))
                    nc.vector.tensor_tensor(
                        outb[u % 2][:, :], t1b[:, :], t2b[:, :],
                        mybir.AluOpType.add)
                    nc.vector.tensor_tensor(
                        wb[:, :], u1b[:, :], u2b[:, :],
                        mybir.AluOpType.add)
                    nc.vector.drain()
                    nc.vector.tensor_tensor(
                        outb[u % 2][:, :], outb[u % 2][:, :], wb[:, :],
                        mybir.AluOpType.subtract)
                    nc.vector.drain().then_inc(wadd, 1)

                for u in range(NUU):
                    if u >= 1:
                        final_add(u - 1)
                    sum_prod(u)
                final_add(NUU - 1)

    nc.compile()
    return nc


# --------------------------------------------------------------------------
# Entry point
# --------------------------------------------------------------------------

def make_consts(W1, b1, W2, b2):
    return {
        "W1t": np.ascontiguousarray(np.asarray(W1, np.float32).T),
        "W2t": np.ascontiguousarray(np.asarray(W2, np.float32).T),
        "b1c": np.asarray(b1, np.float32).reshape(D, 1).copy(),
        "b2c": np.asarray(b2, np.float32).reshape(D, 1).copy(),
        "b1n": (-0.01 * np.asarray(b1, np.float32)).reshape(D, 1).copy(),
        "b2n": (-0.01 * np.asarray(b2, np.float32)).reshape(D, 1).copy(),
    }


def _run(plan, W1, b1, W2, b2, n_cores, debug=False, trace=False):
    nc = build_nc(plan, debug=debug)
    consts = make_consts(W1, b1, W2, b2)
    in_maps = []
    for c in range(n_cores):
        m = dict(plan.inputs[c])
        m.update(consts)
        in_maps.append(m)
    return run_bass_kernel_spmd(nc, in_maps, core_ids=list(range(n_cores)),
                                trace=trace)


def assemble_output(plan, results):
    outs = [np.asarray(results[c]["outT"]).T for c in range(plan.n_cores)]
    return np.concatenate(outs, axis=0)[:plan.n_nodes]


def kernel(entity_embed, att, W1, b1, W2, b2, src, dst):
    entity_embed = np.asarray(entity_embed, np.float32)
    att = np.asarray(att, np.float32)
    src = np.asarray(src).astype(np.int64)
    dst = np.asarray(dst).astype(np.int64)
    plan = make_plan(entity_embed, att, src, dst, n_cores=8)
    res = _run(plan, W1, b1, W2, b2, n_cores=8)
    return assemble_output(plan, res.results)


if __name__ == "__main__":
    pass
